# revision 10
# baseline (speedup 1.0000x reference)
"""Bass/Trainium2 kernel for BoundaryAwareDiceLoss (data-parallel over 8 NeuronCores).

Math (matches the jax reference):
  dice  = 1 - (2*sum(p*t) + 1e-5) / (sum(p) + sum(t) + 1e-5)
  bce   = -mean(t*log(p) + (1-t)*log(1-p))
  bmask = fg & (any of the 6 axis-neighbors (b+-1, h+-1, w+-1), edge-clamped, is bg)
  out   = dice + 10 * bce * mean(bmask)

Key reformulations (t is exactly {0,1}):
  q    = 1 - |p - t|  ->  t*log(p)+(1-t)*log(1-p) = log(q): ONE ACT Ln pass.
  |d|  = bitwise sign-clear of d = p - t (fp16 int16 view, DVE 4x mode).
  S_ad = sum|p-t| = S_p + S_t - 2*S_pt  ->  S_pt derived on host.
  S7   = t + 6 neighbors; non-boundary fg = [S7==7] = relu(S7-6) exactly on ints;
         sum(bmask) = S_t - S_nb.

Boundary stencil on PE with fp8(e4m3) target + DoubleRow perf mode: each DR
matmul sums TWO weighted terms (rhs free dim [j=2, w]) at 0.5 cyc/col:
  per k: (vertical tri + cross-k fix), (extra fix, zero), (b-1,b+1), (w-1,w+1).
W-clamping via padded columns, h-clamping via per-k tridiag weight variants,
b-clamping via host-duplicated halo planes.

Per-core partial sums ([128, 8] f32) are combined on the host in float64.
"""

import os

import numpy as np
import ml_dtypes

F16 = np.float16
F8 = ml_dtypes.float8_e4m3

B_TOTAL, C, H, W = 32, 1, 512, 512
NCORES = 8
B_OWN = B_TOTAL // NCORES  # 4
P = 128
K = H // P  # 4
SLOTS = B_OWN + 2  # 6 (halo_lo, b0..b3, halo_hi)
WB = 520  # [3 filler, lpad, w0..w511, rpad, 3 filler]
D0 = 4  # first data column
TBW = K * SLOTS * WB
PBW = K * B_OWN * W  # 8192
CHW = B_OWN * W  # 2048 per k-chunk
NPIX = float(B_TOTAL * C * H * W)
WEIGHT = 10.0
SMOOTH = 1e-5
PCLIP_LO = 2.0**-11  # min p after host clamp (keeps |p-t| < 1 exactly in fp16)
PCLIP_HI = 1.0 - 2.0**-11  # max p after host clamp (fp16-exact, keeps 1-p > 0)

_CACHE = {}


def _make_weights():
    # matmul computes out[m,n] = sum_p sum_j lhsT[p,j,m] * rhs[p,j,n] (DoubleRow)
    tri = np.zeros((P, P), np.float32)
    for m in range(P):
        tri[m, m] = 1.0
        if m > 0:
            tri[m - 1, m] = 1.0  # out[m] += in[m-1]
        if m < P - 1:
            tri[m + 1, m] = 1.0  # out[m] += in[m+1]
    tri_k0 = tri.copy()
    tri_k0[0, 0] += 1.0  # h-1 clamps to h=0 (self)
    tri_k3 = tri.copy()
    tri_k3[P - 1, P - 1] += 1.0  # h+1 clamps to h=511 (self)
    w_up = np.zeros((P, P), np.float32)
    w_up[P - 1, 0] = 1.0  # out[0] += prev-k-block row 127
    w_dn = np.zeros((P, P), np.float32)
    w_dn[0, P - 1] = 1.0  # out[127] += next-k-block row 0
    eye = np.eye(P, dtype=np.float32)
    zero = np.zeros((P, P), np.float32)
    # DR pairs, indexed q: weights [q, p, j, m] -> stored [p, q, j, m]
    wnp = np.stack([
        np.stack([tri_k0, w_dn], axis=1),  # q0: k=0 vertical (j0=k0, j1=k1)
        np.stack([w_up, tri], axis=1),     # q1: k=1,2 vertical (j0=k-1, j1=k)
        np.stack([w_up, tri_k3], axis=1),  # q2: k=3 vertical (j0=k2, j1=k3)
        np.stack([zero, w_dn], axis=1),    # q3: k=1,2 fix-dn (j0=k, j1=k+1)
        np.stack([eye, eye], axis=1),      # q4: b-pair and w-pair
    ])
    return np.ascontiguousarray(wnp.transpose(1, 0, 2, 3)).astype(F8)


def _build_nc(nrep=1):
    import bass_rust
    import concourse.bacc as bacc
    import concourse.mybir as mybir
    from concourse.tile import TileContext

    dt = mybir.dt
    alu = mybir.AluOpType
    act = mybir.ActivationFunctionType
    DR = mybir.MatmulPerfMode.DoubleRow

    nc = bacc.Bacc("TRN2", target_bir_lowering=False)
    pred_d = nc.dram_tensor("pred", [B_OWN, K, P, W], dt.float16, kind="ExternalInput")
    tgt_d = nc.dram_tensor("target", [SLOTS, K, P, W], dt.float8e4, kind="ExternalInput")
    out_d = nc.dram_tensor("out", [P, 8], dt.float32, kind="ExternalOutput")
    wts_d = nc.inline_tensor(_make_weights(), name="wts")

    with TileContext(nc) as tc:
        with (
            tc.tile_pool(name="big", bufs=1) as big,
            tc.tile_pool(name="ps", bufs=2, space="PSUM") as psp,
        ):
            tb = big.tile([P, TBW], dt.float8e4)
            pb = big.tile([P, PBW], dt.float16)
            dd = big.tile([P, PBW], dt.float16)
            ad = big.tile([P, PBW], dt.float16)
            lnout = big.tile([P, PBW], dt.bfloat16)
            thr = big.tile([P, PBW], dt.bfloat16)
            junk1 = big.tile([P, PBW], dt.bfloat16)
            junk2 = big.tile([P, PBW], dt.bfloat16)
            wsb = big.tile([P, 5 * 2 * P], dt.float8e4)
            # accum columns: Sd 0-3 | S_ad 4-7 | S_p 8-11 | bce 12-15 | nb 16-19
            racc = big.tile([P, 20], dt.float32)
            parts5 = big.tile([P, 5], dt.float32)
            fin = big.tile([P, 8], dt.float32)
            bneg6 = big.tile([P, 1], dt.float32)

            nc.vector.memset(fin[:], 0.0)
            nc.vector.memset(bneg6[:], -6.0)

            wsbv = wsb[:].rearrange("p (q j m) -> p q j m", q=5, j=2)
            tbv = tb[:].rearrange("p (k s wb) -> p k s wb", k=K, s=SLOTS)
            pbv = pb[:].rearrange("p (k x) -> p k x", k=K)
            ddv = dd[:].rearrange("p (k x) -> p k x", k=K)
            adv = ad[:].rearrange("p (k x) -> p k x", k=K)
            lnv = lnout[:].rearrange("p (k x) -> p k x", k=K)
            j1v = junk1[:].rearrange("p (k x) -> p k x", k=K)
            j2v = junk2[:].rearrange("p (k x) -> p k x", k=K)
            ddi = dd[:].bitcast(dt.int16).rearrange("p (k x) -> p k x", k=K)
            adi = ad[:].bitcast(dt.int16).rearrange("p (k x) -> p k x", k=K)

            def wpair_rhs(k, s):
                # [p, j=2 (w-1 | w+1), w=512] overlapping-dim AP
                a = tbv[:, k, s, 3 : 3 + 512].copy()
                off = a.offset
                a.ap = bass_rust.VecI64Pair([[TBW, P], [2, 2], [1, W]])
                a.offset = off
                return a

            for _rep in range(nrep):
                nc.sync.dma_start(
                    out=wsbv, in_=wts_d[:].rearrange("p q j m -> p (q j m)")
                )

                def dma_t(k):
                    nc.sync.dma_start(
                        out=tbv[:, k, :, D0 : D0 + W],
                        in_=tgt_d[:, k].rearrange("s p w -> p s w"),
                    )
                    # w-clamp pads (owned slots only; gpsimd keeps DVE free)
                    nc.gpsimd.tensor_copy(
                        out=tbv[:, k, 1:5, 3:4], in_=tbv[:, k, 1:5, 4:5]
                    )
                    nc.gpsimd.tensor_copy(
                        out=tbv[:, k, 1:5, 516:517], in_=tbv[:, k, 1:5, 515:516]
                    )

                def dma_p(k):
                    nc.sync.dma_start(
                        out=pbv[:, k], in_=pred_d[:, k].rearrange("b p w -> p b w")
                    )

                dma_t(0)
                dma_t(1)
                dma_p(0)
                dma_t(2)
                dma_p(1)
                dma_t(3)
                dma_p(2)
                dma_p(3)

                s7 = [None] * K

                def boundary(k):
                    s7[k] = psp.tile([P, CHW], dt.float32, name="s7", tag="s7")
                    vb = {0: 0, 1: 0, 2: 1, 3: 2}[k]  # vertical rhs base block
                    groups = [(q_vert(k), lambda s: tbv[:, vb : vb + 2, s, D0 : D0 + W])]
                    if k in (1, 2):
                        groups.append(
                            (3, lambda s: tbv[:, k : k + 2, s, D0 : D0 + W])
                        )
                    groups.append((4, lambda s: tbv[:, k, s - 1 : s + 2 : 2, D0 : D0 + W]))
                    groups.append((4, lambda s: wpair_rhs(k, s)))
                    n_g = len(groups)
                    for gi, (q, fn) in enumerate(groups):
                        for s in range(1, 5):
                            nc.tensor.matmul(
                                s7[k][:, (s - 1) * W : s * W],
                                wsbv[:, q],
                                fn(s),
                                start=(gi == 0),
                                stop=(gi == n_g - 1),
                                perf_mode=DR,
                            )

                def q_vert(k):
                    return {0: 0, 1: 1, 2: 1, 3: 2}[k]

                def elementwise(k):
                    t_own = tbv[:, k, 1:5, D0 : D0 + W]
                    # d = p - t ; accum Sd = S_p - S_t
                    nc.vector.scalar_tensor_tensor(
                        out=ddv[:, k], in0=t_own, scalar=-1.0, in1=pbv[:, k],
                        op0=alu.mult, op1=alu.add,
                        accum_out=racc[:, k : k + 1],
                    )
                    # ad = |d| (clear bf16 sign bit; 4x mode)
                    nc.vector.tensor_scalar(
                        out=adi[:, k], in0=ddi[:, k], scalar1=0x7FFF, scalar2=None,
                        op0=alu.bitwise_and,
                    )
                    # S_ad
                    nc.vector.tensor_scalar(
                        out=j1v[:, k], in0=adv[:, k], scalar1=0.0, scalar2=0.0,
                        op0=alu.add, op1=alu.add,
                        accum_out=racc[:, 4 + k : 5 + k],
                    )
                    # S_p
                    nc.vector.tensor_scalar(
                        out=j2v[:, k], in0=pbv[:, k], scalar1=0.0, scalar2=0.0,
                        op0=alu.add, op1=alu.add,
                        accum_out=racc[:, 8 + k : 9 + k],
                    )
                    # bce: sum ln(1 - ad)
                    nc.scalar.activation(
                        out=lnv[:, k], in_=adv[:, k], func=act.Ln,
                        bias=1.0, scale=-1.0,
                        accum_out=racc[:, 12 + k : 13 + k],
                    )

                def threshold(k):
                    # relu(S7-6) == [S7==7] exactly on integers; accum = S_nb
                    nc.scalar.activation(
                        out=thr[:, k * CHW : (k + 1) * CHW], in_=s7[k][:],
                        func=act.Relu, bias=bneg6[:, 0:1], scale=1.0,
                        accum_out=racc[:, 16 + k : 17 + k],
                    )

                for k in range(K):
                    boundary(k)
                    elementwise(k)
                    threshold(k)

                nc.vector.tensor_reduce(
                    out=parts5[:],
                    in_=racc[:].rearrange("p (g n) -> p g n", n=4),
                    axis=mybir.AxisListType.X,
                    op=alu.add,
                )
                nc.vector.tensor_copy(out=fin[:, 0:5], in_=parts5[:])
                nc.sync.dma_start(out=out_d[:], in_=fin[:])

    nc.compile()
    return nc


def _get_nc(nrep=1):
    if nrep not in _CACHE:
        _CACHE[nrep] = _build_nc(nrep)
    return _CACHE[nrep]


def _shard_inputs(pred, target):
    pred = np.asarray(pred, dtype=np.float32).reshape(B_TOTAL, H, W)
    pred_bf = np.clip(pred, PCLIP_LO, PCLIP_HI).astype(F16)
    tgt8 = np.asarray(target, dtype=np.float32).reshape(B_TOTAL, H, W).astype(F8)
    in_maps = []
    for c in range(NCORES):
        b0 = c * B_OWN
        pred_c = np.ascontiguousarray(pred_bf[b0 : b0 + B_OWN]).reshape(
            B_OWN, K, P, W
        )
        lo = max(b0 - 1, 0)
        hi = min(b0 + B_OWN, B_TOTAL - 1)
        idx = [lo] + list(range(b0, b0 + B_OWN)) + [hi]
        tgt_c = np.ascontiguousarray(tgt8[idx]).reshape(SLOTS, K, P, W)
        in_maps.append({"pred": pred_c, "target": tgt_c})
    return in_maps


def _combine(parts_list):
    S = np.zeros(8, dtype=np.float64)
    for r in parts_list:
        S += np.asarray(r, dtype=np.float64).sum(axis=0)
    s_d, s_ad, s_p, s_bce, s_nb = S[0], S[1], S[2], S[3], S[4]
    s_t = s_p - s_d
    s_pt = (s_p + s_t - s_ad) / 2.0
    dice = 1.0 - (2.0 * s_pt + SMOOTH) / (s_p + s_t + SMOOTH)
    bce = -s_bce / NPIX
    mb = (s_t - s_nb) / NPIX
    return np.asarray(dice + WEIGHT * bce * mb, dtype=np.float32)


def kernel(pred, target):
    in_maps = _shard_inputs(pred, target)
    nc = _get_nc()
    if os.environ.get("BASS_SIM") == "1":
        import bass_sim

        res = bass_sim.run_sim(nc, in_maps)
        outs = [r["out"] for r in res]
    else:
        from concourse.bass_utils import run_bass_kernel_spmd

        res = run_bass_kernel_spmd(nc, in_maps, core_ids=list(range(NCORES)))
        outs = [r["out"] for r in res.results]
    return _combine(outs)
